# revision 8
# baseline (speedup 1.0000x reference)
"""BNN MNIST MLP on 8 Trainium2 NeuronCores — pure data parallel.

Model (inference): x[B,784] -> relu(x @ sign(W1)) -> BN1 -> sign ->
@ sign(W2) relu BN2 sign -> @ sign(W3) -> softmax.

Key transformations:
  * BN(relu(h)) >= 0  <=>  h >= t  (per-feature threshold t, since BN scale>0),
    so each binarize step is one ScalarE Sign(h - t) op straight from PSUM.
  * Layer-1 needs fp32-class precision (sign margins ~2.5e-5): x is split on
    host into fp16 hi + lo halves (same total bytes as fp32); both halves are
    stacked into one [1568, B] feature-major tensor and the matmul contracts
    over all 1568 rows against [sign(W1); sign(W1)] — fp16 runs at 1 PE
    cycle/row vs 4 for native fp32, and PSUM accumulates in fp32.
  * x ships pre-transposed (feature-major) per core so the contraction dim
    lands on SBUF partitions with line-rate contiguous DMA; chunks are 128
    partitions wide (full DMA port utilization) and alternate between the
    Sync and Scalar HWDGE rings, prefetched two slabs ahead.
  * The (slab, group) loop is software-pipelined so the PE instruction
    stream never waits on the ScalarE sign ops: L1(t) is emitted before
    L2(t-1) and L3(t-2).
  * Logits [10, 512] are PE-transposed with a stride-16 batch pick so the
    output tile holds 16 consecutive batch rows per partition -> 640 B
    contiguous per partition on the final store (line-rate DMA).
"""
import numpy as np

import concourse.mybir as mybir
from concourse import bacc
from concourse.tile import TileContext
from concourse.bass_utils import run_bass_kernel_spmd

F32 = mybir.dt.float32
F16 = mybir.dt.float16

B = 65536
NCORES = 8
PER = B // NCORES          # 8192 rows per core
SLAB = 1024                # rows per DMA slab
NSLAB = PER // SLAB        # 8
GRP = 512                  # rows per PSUM group (one matmul N)
NGRP = SLAB // GRP         # 2
DSL = 2048                 # rows per transpose/store block (2 slabs)
T = NSLAB * NGRP           # 16 pipeline ticks
K = 784
K2 = 2 * K                 # hi+lo stacked contraction length (1568)
KC = 128                   # contraction chunk (full partition width)
NKC = (K2 + KC - 1) // KC  # 13 chunks: 12 x 128 + 1 x 32
NCLS = 10
NHID = 50
RSTR = DSL // 128          # 16 rows per partition in the output tile

EPS = 1e-3

_CACHE = {}


def _build(prefetch=5, xbufs=6):
    nc = bacc.Bacc("TRN2", target_bir_lowering=False, debug=False,
                   num_devices=NCORES)

    xcat = nc.dram_tensor("xcat", [K2, PER], F16, kind="ExternalInput").ap()
    # all fp16 consts packed in one blob: w1 chunks at cols [50c, 50c+50),
    # w2 at [650, 700), w3 at [700, 710)
    cb16 = nc.dram_tensor("cb16", [128, NHID * NKC + NHID + NCLS], F16,
                          kind="ExternalInput").ap()
    # fp32 consts: col 0 = -T1, col 1 = -T2, cols [2, 12) = identity (rows 0-9)
    cb32 = nc.dram_tensor("cb32", [NHID, 12], F32, kind="ExternalInput").ap()
    out = nc.dram_tensor("out", [PER, NCLS], F32, kind="ExternalOutput").ap()

    kc = [min(KC, K2 - c * KC) for c in range(NKC)]

    with TileContext(nc) as tc:
        with (
            tc.tile_pool(name="consts", bufs=1) as cpool,
            tc.tile_pool(name="xin", bufs=xbufs) as xpool,
            tc.tile_pool(name="mid", bufs=3) as mpool,
            tc.tile_pool(name="fin", bufs=2) as fpool,
            tc.tile_pool(name="psA", bufs=2, space="PSUM") as psA,
            tc.tile_pool(name="psB", bufs=2, space="PSUM") as psB,
        ):
            cb16t = cpool.tile([128, NHID * NKC + NHID + NCLS], F16, tag="cb16")
            nc.sync.dma_start(cb16t[:], cb16[:, :])
            cb32t = cpool.tile([NHID, 12], F32, tag="cb32")
            nc.scalar.dma_start(cb32t[:], cb32[:, :])
            w1t = [cb16t[0:kc[c], c * NHID:(c + 1) * NHID] for c in range(NKC)]
            w2t = cb16t[0:NHID, NKC * NHID:NKC * NHID + NHID]
            w3t = cb16t[0:NHID, NKC * NHID + NHID:NKC * NHID + NHID + NCLS]
            nt1t = cb32t[0:NHID, 0:1]
            nt2t = cb32t[0:NHID, 1:2]
            idt = cb32t[0:NCLS, 2:12]

            xt = {}
            s1t = {}
            s2t = {}
            l3t = {}

            def emit_loads(s):
                b0 = s * SLAB
                xt[s] = []
                for c in range(NKC):
                    t_ = xpool.tile([kc[c], SLAB], F16, tag=f"x_{c}",
                                    name=f"x_{s}_{c}")
                    eng = nc.sync if c % 2 == 0 else nc.scalar
                    eng.dma_start(t_[:], xcat[c * KC:c * KC + kc[c], b0:b0 + SLAB])
                    xt[s].append(t_)

            def stageA(t):
                s, g = divmod(t, NGRP)
                gs = slice(g * GRP, (g + 1) * GRP)
                ps1 = psA.tile([NHID, GRP], F32, tag="ps1")
                for c in range(NKC):
                    nc.tensor.matmul(ps1[:], w1t[c], xt[s][c][:, gs],
                                     start=(c == 0), stop=(c == NKC - 1))
                s1 = mpool.tile([NHID, GRP], F16, tag="s1")
                nc.scalar.sign(s1[:], ps1[:], bias=nt1t)
                s1t[t] = s1

            def stageB(t):
                ps2 = psA.tile([NHID, GRP], F32, tag="ps2")
                nc.tensor.matmul(ps2[:], w2t, s1t[t][:], start=True, stop=True)
                s2 = mpool.tile([NHID, GRP], F16, tag="s2")
                nc.scalar.sign(s2[:], ps2[:], bias=nt2t)
                s2t[t] = s2

            def stageC(t):
                d, q = divmod(t, DSL // GRP)
                qs = slice(q * GRP, (q + 1) * GRP)
                if q == 0:
                    l3t[d] = mpool.tile([NCLS, DSL], F32, tag="l3", name=f"l3_{d}")
                ps3 = psB.tile([NCLS, GRP], F32, tag="ps3")
                nc.tensor.matmul(ps3[:], w3t, s2t[t][:], start=True, stop=True)
                nc.vector.tensor_copy(l3t[d][:, qs], ps3[:])

            def stageD(d):
                b0 = d * DSL
                ps4 = psB.tile([128, RSTR * NCLS], F32, tag="ps4")
                l3v = l3t[d][:].rearrange("c (b r) -> c b r", r=RSTR)
                for r in range(RSTR):
                    nc.tensor.transpose(ps4[:, r * NCLS:(r + 1) * NCLS],
                                        l3v[:, :, r], idt)
                eo = fpool.tile([128, RSTR * NCLS], F32, tag="eo")
                nc.scalar.activation(eo[:], ps4[:],
                                     mybir.ActivationFunctionType.Exp)
                sm = fpool.tile([128, RSTR], F32, tag="sm")
                eov = eo[:].rearrange("p (r c) -> p r c", c=NCLS)
                nc.vector.tensor_reduce(sm[:], eov, axis=mybir.AxisListType.X,
                                        op=mybir.AluOpType.add)
                rv = fpool.tile([128, RSTR], F32, tag="rv")
                nc.vector.reciprocal(rv[:], sm[:])
                ot = fpool.tile([128, RSTR * NCLS], F32, tag="ot")
                otv = ot[:].rearrange("p (r c) -> p r c", c=NCLS)
                rvb = rv[:].unsqueeze(-1).broadcast_to([128, RSTR, NCLS])
                nc.vector.tensor_mul(otv, eov, rvb)
                dst = out[b0:b0 + DSL, :].rearrange("(p r) f -> p (r f)", p=128)
                nc.gpsimd.dma_start(dst, ot[:])

            for s in range(min(prefetch, NSLAB)):
                emit_loads(s)
            for t in range(T + 2):
                if t < T:
                    stageA(t)
                    if t % NGRP == 1 and t // NGRP + prefetch < NSLAB:
                        emit_loads(t // NGRP + prefetch)
                if 0 <= t - 1 < T:
                    stageB(t - 1)
                if 0 <= t - 2 < T:
                    stageC(t - 2)
                    if (t - 2) % (DSL // GRP) == DSL // GRP - 1:
                        stageD((t - 2) // (DSL // GRP))

    nc.compile()
    return nc


def _prep_host(inputs, W1, W2, W3, g1, b1, m1, v1, g2, b2, m2, v2):
    x = np.ascontiguousarray(inputs.reshape(B, K).astype(np.float32, copy=False))
    xhi = x.astype(np.float16)
    xlo = (x - xhi.astype(np.float32)).astype(np.float16)

    w1b = np.where(W1 >= 0, 1.0, -1.0).astype(np.float16)
    w2b = np.where(W2 >= 0, 1.0, -1.0).astype(np.float16)
    w3b = np.where(W3 >= 0, 1.0, -1.0).astype(np.float16)

    a1 = g1.astype(np.float64) / np.sqrt(v1.astype(np.float64) + EPS)
    c1 = b1.astype(np.float64) - a1 * m1.astype(np.float64)
    t1 = -c1 / a1
    T1 = np.where(t1 > 0, t1, -1e30).astype(np.float32)
    a2 = g2.astype(np.float64) / np.sqrt(v2.astype(np.float64) + EPS)
    c2 = b2.astype(np.float64) - a2 * m2.astype(np.float64)
    t2 = -c2 / a2
    T2 = np.where(t2 > 0, t2, -1e30).astype(np.float32)

    w1cat = np.vstack([w1b, w1b])
    cb16 = np.zeros((128, NHID * NKC + NHID + NCLS), dtype=np.float16)
    for c in range(NKC):
        n = min(KC, K2 - c * KC)
        cb16[:n, c * NHID:(c + 1) * NHID] = w1cat[c * KC:c * KC + n]
    cb16[:NHID, NKC * NHID:NKC * NHID + NHID] = w2b
    cb16[:NHID, NKC * NHID + NHID:] = w3b
    cb32 = np.zeros((NHID, 12), dtype=np.float32)
    cb32[:, 0] = -T1
    cb32[:, 1] = -T2
    cb32[:NCLS, 2:12] = np.eye(NCLS, dtype=np.float32)
    shared = {"cb16": cb16, "cb32": cb32}
    in_maps = []
    for c in range(NCORES):
        sl = slice(c * PER, (c + 1) * PER)
        m = dict(shared)
        xc = np.empty((K2, PER), dtype=np.float16)
        xc[:K] = xhi[sl].T
        xc[K:] = xlo[sl].T
        m["xcat"] = xc
        in_maps.append(m)
    return in_maps


def kernel(**inputs):
    if "nc" not in _CACHE:
        _CACHE["nc"] = _build()
    nc = _CACHE["nc"]
    in_maps = _prep_host(**inputs)
    res = run_bass_kernel_spmd(nc, in_maps, core_ids=list(range(NCORES)))
    return np.concatenate([r["out"] for r in res.results], axis=0)


# revision 9
# speedup vs baseline: 1.0208x; 1.0208x over previous
"""BNN MNIST MLP on 8 Trainium2 NeuronCores — pure data parallel.

Model (inference): x[B,784] -> relu(x @ sign(W1)) -> BN1 -> sign ->
@ sign(W2) relu BN2 sign -> @ sign(W3) -> softmax.

Key transformations:
  * BN(relu(h)) >= 0  <=>  h >= t  (per-feature threshold t, since BN scale>0),
    so each binarize step is one ScalarE Sign(h - t) op straight from PSUM.
  * Layer-1 needs fp32-class precision (sign margins ~2.5e-5): x is split on
    host into fp16 hi + lo halves (same total bytes as fp32); both halves are
    stacked into one [1568, B] feature-major tensor and the matmul contracts
    over all 1568 rows against [sign(W1); sign(W1)] — fp16 runs at 1 PE
    cycle/row vs 4 for native fp32, and PSUM accumulates in fp32.
  * x ships pre-transposed (feature-major) per core so the contraction dim
    lands on SBUF partitions with line-rate contiguous DMA; chunks are 128
    partitions wide (full DMA port utilization) and alternate between the
    Sync and Scalar HWDGE rings, prefetched two slabs ahead.
  * The (slab, group) loop is software-pipelined so the PE instruction
    stream never waits on the ScalarE sign ops: L1(t) is emitted before
    L2(t-1) and L3(t-2).
  * Logits [10, 512] are PE-transposed with a stride-16 batch pick so the
    output tile holds 16 consecutive batch rows per partition -> 640 B
    contiguous per partition on the final store (line-rate DMA).
"""
import numpy as np

import concourse.mybir as mybir
from concourse import bacc
from concourse.tile import TileContext
from concourse.bass_utils import run_bass_kernel_spmd

F32 = mybir.dt.float32
F16 = mybir.dt.float16

B = 65536
NCORES = 8
PER = B // NCORES          # 8192 rows per core
SLAB = 1024                # rows per DMA slab
NSLAB = PER // SLAB        # 8
GRP = 512                  # rows per PSUM group (one matmul N)
NGRP = SLAB // GRP         # 2
DSL = 2048                 # rows per transpose/store block (2 slabs)
T = NSLAB * NGRP           # 16 pipeline ticks
K = 784
K2 = 2 * K                 # hi+lo stacked contraction length (1568)
KC = 128                   # contraction chunk (full partition width)
NKC = (K2 + KC - 1) // KC  # 13 chunks: 12 x 128 + 1 x 32
NCLS = 10
NHID = 50
RSTR = DSL // 128          # 16 rows per partition in the output tile

EPS = 1e-3

_CACHE = {}


def _build(prefetch=5, xbufs=6):
    nc = bacc.Bacc("TRN2", target_bir_lowering=False, debug=False,
                   num_devices=NCORES)

    xcat = nc.dram_tensor("xcat", [K2, PER], F16, kind="ExternalInput").ap()
    # all fp16 consts packed in one blob: w1 chunks at cols [50c, 50c+50),
    # w2 at [650, 700), w3 at [700, 710)
    cb16 = nc.dram_tensor("cb16", [128, NHID * NKC + NHID + NCLS], F16,
                          kind="ExternalInput").ap()
    # fp32 consts: col 0 = -T1, col 1 = -T2, cols [2, 12) = identity (rows 0-9)
    cb32 = nc.dram_tensor("cb32", [NHID, 12], F32, kind="ExternalInput").ap()
    out = nc.dram_tensor("out", [PER, NCLS], F32, kind="ExternalOutput").ap()

    kc = [min(KC, K2 - c * KC) for c in range(NKC)]

    with TileContext(nc) as tc:
        with (
            tc.tile_pool(name="consts", bufs=1) as cpool,
            tc.tile_pool(name="xin", bufs=xbufs) as xpool,
            tc.tile_pool(name="mid", bufs=3) as mpool,
            tc.tile_pool(name="fin", bufs=2) as fpool,
            tc.tile_pool(name="psA", bufs=2, space="PSUM") as psA,
            tc.tile_pool(name="psB", bufs=2, space="PSUM") as psB,
        ):
            cb16t = cpool.tile([128, NHID * NKC + NHID + NCLS], F16, tag="cb16")
            nc.sync.dma_start(cb16t[:], cb16[:, :])
            cb32t = cpool.tile([NHID, 12], F32, tag="cb32")
            nc.scalar.dma_start(cb32t[:], cb32[:, :])
            w1t = [cb16t[0:kc[c], c * NHID:(c + 1) * NHID] for c in range(NKC)]
            w2t = cb16t[0:NHID, NKC * NHID:NKC * NHID + NHID]
            w3t = cb16t[0:NHID, NKC * NHID + NHID:NKC * NHID + NHID + NCLS]
            nt1t = cb32t[0:NHID, 0:1]
            nt2t = cb32t[0:NHID, 1:2]
            idt = cb32t[0:NCLS, 2:12]

            xt = {}
            s1t = {}
            s2t = {}
            l3t = {}

            def emit_loads(s):
                b0 = s * SLAB
                xt[s] = []
                for c in range(NKC):
                    t_ = xpool.tile([kc[c], SLAB], F16, tag=f"x_{c}",
                                    name=f"x_{s}_{c}")
                    eng = nc.sync if c % 2 == 0 else nc.scalar
                    eng.dma_start(t_[:], xcat[c * KC:c * KC + kc[c], b0:b0 + SLAB])
                    xt[s].append(t_)

            def stageA(t):
                s, g = divmod(t, NGRP)
                gs = slice(g * GRP, (g + 1) * GRP)
                ps1 = psA.tile([NHID, GRP], F32, tag="ps1")
                for c in range(NKC):
                    nc.tensor.matmul(ps1[:], w1t[c], xt[s][c][:, gs],
                                     start=(c == 0), stop=(c == NKC - 1))
                s1 = mpool.tile([NHID, GRP], F16, tag="s1")
                nc.scalar.sign(s1[:], ps1[:], bias=nt1t)
                s1t[t] = s1

            def stageB(t):
                ps2 = psA.tile([NHID, GRP], F32, tag="ps2")
                nc.tensor.matmul(ps2[:], w2t, s1t[t][:], start=True, stop=True)
                s2 = mpool.tile([NHID, GRP], F16, tag="s2")
                nc.scalar.sign(s2[:], ps2[:], bias=nt2t)
                s2t[t] = s2

            def stageC(t):
                d, q = divmod(t, DSL // GRP)
                qs = slice(q * GRP, (q + 1) * GRP)
                if q == 0:
                    l3t[d] = mpool.tile([NCLS, DSL], F32, tag="l3", name=f"l3_{d}")
                ps3 = psB.tile([NCLS, GRP], F32, tag="ps3")
                nc.tensor.matmul(ps3[:], w3t, s2t[t][:], start=True, stop=True)
                nc.vector.tensor_copy(l3t[d][:, qs], ps3[:])

            def stageD(d):
                b0 = d * DSL
                ps4 = psB.tile([128, RSTR * NCLS], F32, tag="ps4")
                l3v = l3t[d][:].rearrange("c (b r) -> c b r", r=RSTR)
                for r in range(RSTR):
                    nc.tensor.transpose(ps4[:, r * NCLS:(r + 1) * NCLS],
                                        l3v[:, :, r], idt)
                eo = fpool.tile([128, RSTR * NCLS], F32, tag="eo")
                nc.scalar.activation(eo[:], ps4[:],
                                     mybir.ActivationFunctionType.Exp)
                sm = fpool.tile([128, RSTR], F32, tag="sm")
                eov = eo[:].rearrange("p (r c) -> p r c", c=NCLS)
                nc.vector.tensor_reduce(sm[:], eov, axis=mybir.AxisListType.X,
                                        op=mybir.AluOpType.add)
                rv = fpool.tile([128, RSTR], F32, tag="rv")
                nc.vector.reciprocal(rv[:], sm[:])
                ot = fpool.tile([128, RSTR * NCLS], F32, tag="ot")
                otv = ot[:].rearrange("p (r c) -> p r c", c=NCLS)
                rvb = rv[:].unsqueeze(-1).broadcast_to([128, RSTR, NCLS])
                nc.vector.tensor_mul(otv, eov, rvb)
                dst = out[b0:b0 + DSL, :].rearrange("(p r) f -> p (r f)", p=128)
                nc.sync.dma_start(dst, ot[:])

            for s in range(min(prefetch, NSLAB)):
                emit_loads(s)
            for t in range(T + 2):
                if t < T:
                    stageA(t)
                    if t % NGRP == 1 and t // NGRP + prefetch < NSLAB:
                        emit_loads(t // NGRP + prefetch)
                if 0 <= t - 1 < T:
                    stageB(t - 1)
                if 0 <= t - 2 < T:
                    stageC(t - 2)
                    if (t - 2) % (DSL // GRP) == DSL // GRP - 1:
                        stageD((t - 2) // (DSL // GRP))

    nc.compile()
    return nc


def _prep_host(inputs, W1, W2, W3, g1, b1, m1, v1, g2, b2, m2, v2):
    x = np.ascontiguousarray(inputs.reshape(B, K).astype(np.float32, copy=False))
    xhi = x.astype(np.float16)
    xlo = (x - xhi.astype(np.float32)).astype(np.float16)

    w1b = np.where(W1 >= 0, 1.0, -1.0).astype(np.float16)
    w2b = np.where(W2 >= 0, 1.0, -1.0).astype(np.float16)
    w3b = np.where(W3 >= 0, 1.0, -1.0).astype(np.float16)

    a1 = g1.astype(np.float64) / np.sqrt(v1.astype(np.float64) + EPS)
    c1 = b1.astype(np.float64) - a1 * m1.astype(np.float64)
    t1 = -c1 / a1
    T1 = np.where(t1 > 0, t1, -1e30).astype(np.float32)
    a2 = g2.astype(np.float64) / np.sqrt(v2.astype(np.float64) + EPS)
    c2 = b2.astype(np.float64) - a2 * m2.astype(np.float64)
    t2 = -c2 / a2
    T2 = np.where(t2 > 0, t2, -1e30).astype(np.float32)

    w1cat = np.vstack([w1b, w1b])
    cb16 = np.zeros((128, NHID * NKC + NHID + NCLS), dtype=np.float16)
    for c in range(NKC):
        n = min(KC, K2 - c * KC)
        cb16[:n, c * NHID:(c + 1) * NHID] = w1cat[c * KC:c * KC + n]
    cb16[:NHID, NKC * NHID:NKC * NHID + NHID] = w2b
    cb16[:NHID, NKC * NHID + NHID:] = w3b
    cb32 = np.zeros((NHID, 12), dtype=np.float32)
    cb32[:, 0] = -T1
    cb32[:, 1] = -T2
    cb32[:NCLS, 2:12] = np.eye(NCLS, dtype=np.float32)
    shared = {"cb16": cb16, "cb32": cb32}
    in_maps = []
    for c in range(NCORES):
        sl = slice(c * PER, (c + 1) * PER)
        m = dict(shared)
        xc = np.empty((K2, PER), dtype=np.float16)
        xc[:K] = xhi[sl].T
        xc[K:] = xlo[sl].T
        m["xcat"] = xc
        in_maps.append(m)
    return in_maps


def kernel(**inputs):
    if "nc" not in _CACHE:
        _CACHE["nc"] = _build()
    nc = _CACHE["nc"]
    in_maps = _prep_host(**inputs)
    res = run_bass_kernel_spmd(nc, in_maps, core_ids=list(range(NCORES)))
    return np.concatenate([r["out"] for r in res.results], axis=0)


# revision 10
# speedup vs baseline: 1.0773x; 1.0553x over previous
"""BNN MNIST MLP on 8 Trainium2 NeuronCores — pure data parallel.

Model (inference): x[B,784] -> relu(x @ sign(W1)) -> BN1 -> sign ->
@ sign(W2) relu BN2 sign -> @ sign(W3) -> softmax.

Key transformations:
  * BN(relu(h)) >= 0  <=>  h >= t  (per-feature threshold t, since BN scale>0),
    so each binarize step is one ScalarE Sign(h - t) op straight from PSUM.
  * Layer-1 needs fp32-class precision (sign margins ~2.5e-5): x is split on
    host into fp16 hi + lo halves (same total bytes as fp32); both halves are
    stacked into one [1568, B] feature-major tensor and the matmul contracts
    over all 1568 rows against [sign(W1); sign(W1)] — fp16 runs at 1 PE
    cycle/row vs 4 for native fp32, and PSUM accumulates in fp32.
  * x ships pre-transposed (feature-major) per core so the contraction dim
    lands on SBUF partitions with line-rate contiguous DMA; chunks are 128
    partitions wide (full DMA port utilization) and alternate between the
    Sync and Scalar HWDGE rings, prefetched two slabs ahead.
  * The (slab, group) loop is software-pipelined so the PE instruction
    stream never waits on the ScalarE sign ops: L1(t) is emitted before
    L2(t-1) and L3(t-2).
  * Logits [10, 512] are PE-transposed with a stride-16 batch pick so the
    output tile holds 16 consecutive batch rows per partition -> 640 B
    contiguous per partition on the final store (line-rate DMA).
"""
import numpy as np

import concourse.mybir as mybir
from concourse import bacc
from concourse.tile import TileContext
from concourse.bass_utils import run_bass_kernel_spmd

F32 = mybir.dt.float32
F16 = mybir.dt.float16

B = 65536
NCORES = 8
PER = B // NCORES          # 8192 rows per core
SLAB = 1024                # rows per DMA slab
NSLAB = PER // SLAB        # 8
GRP = 512                  # rows per PSUM group (one matmul N)
NGRP = SLAB // GRP         # 2
DSL = 2048                 # rows per transpose/store block (2 slabs)
T = NSLAB * NGRP           # 16 pipeline ticks
K = 784
K2 = 2 * K                 # hi+lo stacked contraction length (1568)
KC = 128                   # contraction chunk (full partition width)
NKC = (K2 + KC - 1) // KC  # 13 chunks: 12 x 128 + 1 x 32
NCLS = 10
NHID = 50
RSTR = DSL // 128          # 16 rows per partition in the output tile

EPS = 1e-3

_CACHE = {}


def _build(prefetch=4, xbufs=5):
    nc = bacc.Bacc("TRN2", target_bir_lowering=False, debug=False,
                   num_devices=NCORES)

    xcat = nc.dram_tensor("xcat", [K2, PER], F16, kind="ExternalInput").ap()
    # all fp16 consts packed in one blob: w1 chunks at cols [50c, 50c+50),
    # w2 at [650, 700), w3 at [700, 710)
    cb16 = nc.dram_tensor("cb16", [128, NHID * NKC + NHID + NCLS], F16,
                          kind="ExternalInput").ap()
    # fp32 consts: col 0 = -T1, col 1 = -T2, cols [2, 12) = identity (rows 0-9)
    cb32 = nc.dram_tensor("cb32", [NHID, 12], F32, kind="ExternalInput").ap()
    out = nc.dram_tensor("out", [PER, NCLS], F32, kind="ExternalOutput").ap()

    kc = [min(KC, K2 - c * KC) for c in range(NKC)]

    with TileContext(nc) as tc:
        with (
            tc.tile_pool(name="consts", bufs=1) as cpool,
            tc.tile_pool(name="xin", bufs=xbufs) as xpool,
            tc.tile_pool(name="mid", bufs=3) as mpool,
            tc.tile_pool(name="fin", bufs=2) as fpool,
            tc.tile_pool(name="psA", bufs=2, space="PSUM") as psA,
            tc.tile_pool(name="psB", bufs=2, space="PSUM") as psB,
        ):
            cb16t = cpool.tile([128, NHID * NKC + NHID + NCLS], F16, tag="cb16")
            nc.sync.dma_start(cb16t[:], cb16[:, :])
            cb32t = cpool.tile([NHID, 12], F32, tag="cb32")
            nc.scalar.dma_start(cb32t[:], cb32[:, :])
            w1t = [cb16t[0:kc[c], c * NHID:(c + 1) * NHID] for c in range(NKC)]
            w2t = cb16t[0:NHID, NKC * NHID:NKC * NHID + NHID]
            w3t = cb16t[0:NHID, NKC * NHID + NHID:NKC * NHID + NHID + NCLS]
            nt1t = cb32t[0:NHID, 0:1]
            nt2t = cb32t[0:NHID, 1:2]
            idt = cb32t[0:NCLS, 2:12]

            xt = {}
            s1t = {}
            s2t = {}
            l3t = {}

            def emit_loads(s):
                b0 = s * SLAB
                xt[s] = []
                for c in range(NKC):
                    t_ = xpool.tile([kc[c], SLAB], F16, tag=f"x_{c}",
                                    name=f"x_{s}_{c}")
                    eng = nc.sync if c % 2 == 0 else nc.scalar
                    eng.dma_start(t_[:], xcat[c * KC:c * KC + kc[c], b0:b0 + SLAB])
                    xt[s].append(t_)

            def stageA(t):
                s, g = divmod(t, NGRP)
                gs = slice(g * GRP, (g + 1) * GRP)
                ps1 = psA.tile([NHID, GRP], F32, tag="ps1")
                for c in range(NKC):
                    nc.tensor.matmul(ps1[:], w1t[c], xt[s][c][:, gs],
                                     start=(c == 0), stop=(c == NKC - 1))
                s1 = mpool.tile([NHID, GRP], F16, tag="s1")
                nc.scalar.sign(s1[:], ps1[:], bias=nt1t)
                s1t[t] = s1

            def stageB(t):
                ps2 = psA.tile([NHID, GRP], F32, tag="ps2")
                nc.tensor.matmul(ps2[:], w2t, s1t[t][:], start=True, stop=True)
                s2 = mpool.tile([NHID, GRP], F16, tag="s2")
                nc.scalar.sign(s2[:], ps2[:], bias=nt2t)
                s2t[t] = s2

            def stageC(t):
                d, q = divmod(t, DSL // GRP)
                qs = slice(q * GRP, (q + 1) * GRP)
                if q == 0:
                    l3t[d] = mpool.tile([NCLS, DSL], F32, tag="l3", name=f"l3_{d}")
                ps3 = psB.tile([NCLS, GRP], F32, tag="ps3")
                nc.tensor.matmul(ps3[:], w3t, s2t[t][:], start=True, stop=True)
                nc.vector.tensor_copy(l3t[d][:, qs], ps3[:])

            def stageD(d):
                b0 = d * DSL
                ps4 = psB.tile([128, RSTR * NCLS], F32, tag="ps4")
                l3v = l3t[d][:].rearrange("c (b r) -> c b r", r=RSTR)
                for r in range(RSTR):
                    nc.tensor.transpose(ps4[:, r * NCLS:(r + 1) * NCLS],
                                        l3v[:, :, r], idt)
                eo = fpool.tile([128, RSTR * NCLS], F32, tag="eo")
                nc.scalar.activation(eo[:], ps4[:],
                                     mybir.ActivationFunctionType.Exp)
                sm = fpool.tile([128, RSTR], F32, tag="sm")
                eov = eo[:].rearrange("p (r c) -> p r c", c=NCLS)
                nc.vector.tensor_reduce(sm[:], eov, axis=mybir.AxisListType.X,
                                        op=mybir.AluOpType.add)
                rv = fpool.tile([128, RSTR], F32, tag="rv")
                nc.vector.reciprocal(rv[:], sm[:])
                ot = fpool.tile([128, RSTR * NCLS], F32, tag="ot")
                otv = ot[:].rearrange("p (r c) -> p r c", c=NCLS)
                rvb = rv[:].unsqueeze(-1).broadcast_to([128, RSTR, NCLS])
                nc.vector.tensor_mul(otv, eov, rvb)
                dst = out[b0:b0 + DSL, :].rearrange("(p r) f -> p (r f)", p=128)
                nc.sync.dma_start(dst, ot[:])

            for s in range(min(prefetch, NSLAB)):
                emit_loads(s)
            for t in range(T + 2):
                if t < T:
                    stageA(t)
                    if t % NGRP == 1 and t // NGRP + prefetch < NSLAB:
                        emit_loads(t // NGRP + prefetch)
                if 0 <= t - 1 < T:
                    stageB(t - 1)
                if 0 <= t - 2 < T:
                    stageC(t - 2)
                    if (t - 2) % (DSL // GRP) == DSL // GRP - 1:
                        stageD((t - 2) // (DSL // GRP))

    nc.compile()
    return nc


def _prep_host(inputs, W1, W2, W3, g1, b1, m1, v1, g2, b2, m2, v2):
    x = np.ascontiguousarray(inputs.reshape(B, K).astype(np.float32, copy=False))
    xhi = x.astype(np.float16)
    xlo = (x - xhi.astype(np.float32)).astype(np.float16)

    w1b = np.where(W1 >= 0, 1.0, -1.0).astype(np.float16)
    w2b = np.where(W2 >= 0, 1.0, -1.0).astype(np.float16)
    w3b = np.where(W3 >= 0, 1.0, -1.0).astype(np.float16)

    a1 = g1.astype(np.float64) / np.sqrt(v1.astype(np.float64) + EPS)
    c1 = b1.astype(np.float64) - a1 * m1.astype(np.float64)
    t1 = -c1 / a1
    T1 = np.where(t1 > 0, t1, -1e30).astype(np.float32)
    a2 = g2.astype(np.float64) / np.sqrt(v2.astype(np.float64) + EPS)
    c2 = b2.astype(np.float64) - a2 * m2.astype(np.float64)
    t2 = -c2 / a2
    T2 = np.where(t2 > 0, t2, -1e30).astype(np.float32)

    w1cat = np.vstack([w1b, w1b])
    cb16 = np.zeros((128, NHID * NKC + NHID + NCLS), dtype=np.float16)
    for c in range(NKC):
        n = min(KC, K2 - c * KC)
        cb16[:n, c * NHID:(c + 1) * NHID] = w1cat[c * KC:c * KC + n]
    cb16[:NHID, NKC * NHID:NKC * NHID + NHID] = w2b
    cb16[:NHID, NKC * NHID + NHID:] = w3b
    cb32 = np.zeros((NHID, 12), dtype=np.float32)
    cb32[:, 0] = -T1
    cb32[:, 1] = -T2
    cb32[:NCLS, 2:12] = np.eye(NCLS, dtype=np.float32)
    shared = {"cb16": cb16, "cb32": cb32}
    in_maps = []
    for c in range(NCORES):
        sl = slice(c * PER, (c + 1) * PER)
        m = dict(shared)
        xc = np.empty((K2, PER), dtype=np.float16)
        xc[:K] = xhi[sl].T
        xc[K:] = xlo[sl].T
        m["xcat"] = xc
        in_maps.append(m)
    return in_maps


def kernel(**inputs):
    if "nc" not in _CACHE:
        _CACHE["nc"] = _build()
    nc = _CACHE["nc"]
    in_maps = _prep_host(**inputs)
    res = run_bass_kernel_spmd(nc, in_maps, core_ids=list(range(NCORES)))
    return np.concatenate([r["out"] for r in res.results], axis=0)
